# revision 21
# baseline (speedup 1.0000x reference)
"""GCNN message-passing kernel for 8 Trainium2 NeuronCores.

Sharding: adjacency rows (atom dim) are sharded across 8 cores (1024
atoms each); node features h are re-replicated via an fp8 AllGather
after every layer; the small readout MLP runs redundantly on every
core after an AllReduce of the per-shard atom sums.

Key layout choice: the host feeds each core its adjacency shard
PRE-TRANSPOSED and pre-biased as  cmask[j, i] = 128*(A[i, j] - 1)
in fp16 (values {0, -128}).  With j on partitions this is exactly the
orientation the PE needs for the aggregation matmuls (contraction over
j), so the kernel performs no transposes of A at all, and the masked
attention becomes

    M^T[j, i] = sigmoid( cmask[j, i] + s_nei[j] + s_self[i] + b )

(sigmoid(-128+x) == 0 in fp8/bf16).  Per j-tile the two rank-1 terms
are folded in one DVE scalar_tensor_tensor (s_nei as the per-partition
scalar column, s_self as a broadcast fp16 row) at 2x rate, and the
sigmoid runs on the scalar engine over 8-tile chunks (amortizing the
~224-cycle per-instruction overhead), bias-free.

The mask and the gathered h payload are fp8e4, so the aggregation
matmuls run in DoubleRow perf mode (2 fp8 MACs/cell/cycle), contracting
j-tile pairs [128, 2, ...].  agg^T[h', i] = sum_j h[j, h'] M^T[j, i]
keeps everything transpose-free downstream; W_node matmuls run in bf16.

The payload row per atom is [h fp8 | s_nei_hi fp8 | s_nei_lo fp8 | pad]
-- s_nei ships as a two-term fp8 sum to keep logit error ~1e-2.
"""

import numpy as np

import concourse.bass as bass
import concourse.bacc as bacc
import concourse.mybir as mybir
import concourse.tile as tile
from concourse.bass_utils import run_bass_kernel_spmd
from concourse.masks import make_identity

FP32 = mybir.dt.float32
BF16 = mybir.dt.bfloat16
FP16 = mybir.dt.float16
FP8 = mybir.dt.float8e4
AF = mybir.ActivationFunctionType
ALU = mybir.AluOpType
DR = mybir.MatmulPerfMode.DoubleRow

P = 128
NEG = -128.0  # mask offset: sigmoid(-128 + logit) == 0 for any sane logit


def build_gcnn(N=8192, F=133, H=256, MF=200, RH=512, DEPTH=3, N_RO=2, n_cores=8,
               res_tiles=None, sig_ch=None, no_collectives=False):
    S = N // n_cores          # atoms per core (row shard)
    JT = N // P               # j (neighbor) tiles over the full atom dim
    IT = S // P               # i tiles within the shard
    IC = 512 if S % 512 == 0 else S   # psum free-dim chunk
    NIC = S // IC
    HC = H // P               # hidden chunks of 128
    PAY = H + 16              # fp8 payload row: h | s_hi | s_lo | pad to %16
    RT = RH // P
    SIG = sig_ch or min(8, JT)          # j-tiles per sigmoid chunk
    NCH = JT // SIG
    # number of cmask j-tiles kept resident in SBUF (rest streamed per layer)
    RES = res_tiles if res_tiles is not None else (16 if JT > 24 else JT)
    SC = 4                    # streamed-tile DMA chunk
    assert (JT - RES) % SC == 0 or RES == JT
    rg = [list(range(n_cores))]

    nc = bacc.Bacc("TRN2", target_bir_lowering=False, debug=False,
                   num_devices=n_cores)

    # ---------------- I/O ----------------
    CM = nc.dram_tensor("cmask", [N, S], FP16, kind="ExternalInput")
    XT = nc.dram_tensor("atom_t", [F, S], BF16, kind="ExternalInput")
    MOLV = nc.dram_tensor("mol", [1, MF], FP32, kind="ExternalInput")
    WIN = nc.dram_tensor("w_in", [F, H], BF16, kind="ExternalInput")
    BIN = nc.dram_tensor("b_in", [1, H], BF16, kind="ExternalInput")
    WATT = nc.dram_tensor("w_att", [2, H], FP32, kind="ExternalInput")
    BATT = nc.dram_tensor("b_att", [1, 1], FP32, kind="ExternalInput")
    WNODE = nc.dram_tensor("w_node", [DEPTH, H, H], BF16, kind="ExternalInput")
    BNODE = nc.dram_tensor("b_node", [1, DEPTH * H], BF16, kind="ExternalInput")
    WROIN = nc.dram_tensor("w_ro_in", [H + MF, RH], FP32, kind="ExternalInput")
    BROIN = nc.dram_tensor("b_ro_in", [1, RH], FP32, kind="ExternalInput")
    WROH = nc.dram_tensor("w_ro_hid", [N_RO, RH, RH], FP32, kind="ExternalInput")
    BROH = nc.dram_tensor("b_ro_hid", [1, N_RO * RH], FP32, kind="ExternalInput")
    WOUT = nc.dram_tensor("w_out", [RH, 1], FP32, kind="ExternalInput")
    BOUT = nc.dram_tensor("b_out", [1, 1], FP32, kind="ExternalInput")
    OUT = nc.dram_tensor("out", [1, 1], FP32, kind="ExternalOutput")

    with tile.TileContext(nc) as tc:
        _build_body(nc, tc, locals())
    nc.compile()
    return nc


def _build_body(nc, tc, v):
    N, F, H, MF, RH, DEPTH, N_RO = (v[k] for k in
                                    ("N", "F", "H", "MF", "RH", "DEPTH", "N_RO"))
    S, JT, IT, IC, NIC, HC, PAY = (v[k] for k in
                                   ("S", "JT", "IT", "IC", "NIC", "HC", "PAY"))
    RT, SIG, NCH, RES, SC, rg = (v[k] for k in
                                 ("RT", "SIG", "NCH", "RES", "SC", "rg"))
    CM, XT, MOLV, WIN, BIN, WATT, BATT = (v[k] for k in
                                          ("CM", "XT", "MOLV", "WIN", "BIN",
                                           "WATT", "BATT"))
    WNODE, BNODE, WROIN, BROIN, WROH, BROH, WOUT, BOUT, OUT = (
        v[k] for k in ("WNODE", "BNODE", "WROIN", "BROIN", "WROH", "BROH",
                       "WOUT", "BOUT", "OUT"))
    CORES = v["n_cores"]

    import contextlib
    ctx = contextlib.ExitStack()
    with ctx:
        consts = ctx.enter_context(tc.tile_pool(name="consts", bufs=1))
        work = ctx.enter_context(tc.tile_pool(name="work", bufs=3))
        big = ctx.enter_context(tc.tile_pool(name="big", bufs=1))
        gbuf = ctx.enter_context(tc.tile_pool(name="gbuf", bufs=2))
        strm = ctx.enter_context(tc.tile_pool(name="strm", bufs=2))
        t2p = ctx.enter_context(tc.tile_pool(name="t2p", bufs=2))
        m8p = ctx.enter_context(tc.tile_pool(name="m8p", bufs=2))
        pacc = ctx.enter_context(tc.tile_pool(name="pacc", bufs=1, space="PSUM"))
        pcyc = ctx.enter_context(tc.tile_pool(name="pcyc", bufs=3, space="PSUM"))
        dram = ctx.enter_context(tc.tile_pool(name="dram", bufs=1, space="DRAM"))

        # ------------- DRAM scratch -------------
        pays = [dram.tile([P, IT * PAY], FP8, name=f"pay{d}")
                for d in range(DEPTH)]
        Gs = [dram.tile([CORES * P, IT * PAY], FP8, addr_space="Shared",
                        name=f"gath{d}")
              for d in range(DEPTH)]
        ar_in = dram.tile([1, H], FP32, name="ar_in")
        ar_out = dram.tile([1, H], FP32, addr_space="Shared", name="ar_out")

        # ------------- constants / weights -------------
        ones_bf = consts.tile([P, P], BF16, name="ones_bf")
        nc.gpsimd.memset(ones_bf[:], 1.0)
        ones_f = consts.tile([P, P], FP32, name="ones_f")
        nc.gpsimd.memset(ones_f[:], 1.0)
        ident_f = consts.tile([P, P], FP32, name="ident_f")
        make_identity(nc, ident_f[:])

        # input weights (f on partitions, two chunks if F > 128)
        fw2 = F - P if F > P else 0
        win_sb = consts.tile([P, H], BF16, name="win_sb")
        nc.sync.dma_start(out=win_sb[:], in_=WIN[0:P, :])
        if fw2:
            win2_sb = consts.tile([fw2, H], BF16, name="win2_sb")
            nc.sync.dma_start(out=win2_sb[:], in_=WIN[P:F, :])
        bin_sb = consts.tile([1, H], BF16, name="bin_sb")
        nc.sync.dma_start(out=bin_sb[:], in_=BIN[:])
        xt_sb = consts.tile([P, S], BF16, name="xt_sb")
        nc.sync.dma_start(out=xt_sb[:], in_=XT[0:P, :])
        if fw2:
            xt2_sb = consts.tile([fw2, S], BF16, name="xt2_sb")
            nc.sync.dma_start(out=xt2_sb[:], in_=XT[P:F, :])

        wnode_sb = consts.tile([P, DEPTH, HC, H], BF16, name="wnode_sb")
        nc.sync.dma_start(out=wnode_sb[:],
                          in_=WNODE.rearrange("d (kt p) h -> p d kt h", p=P))
        bnode_sb = consts.tile([1, DEPTH * H], BF16, name="bnode_sb")
        nc.sync.dma_start(out=bnode_sb[:], in_=BNODE[:])

        watt_sb = consts.tile([1, 2 * H], FP32, name="watt_sb")
        nc.sync.dma_start(out=watt_sb[0:1, 0:H], in_=WATT[0:1, :])
        nc.sync.dma_start(out=watt_sb[0:1, H:2 * H], in_=WATT[1:2, :])
        batt_sb = consts.tile([1, 1], FP32, name="batt_sb")
        nc.sync.dma_start(out=batt_sb[:], in_=BATT[:])

        # readout weights, fp32 (keeps final-MLP precision high)
        wro_sb = consts.tile([P, 4, RH], FP32, name="wro_sb")
        nc.vector.memset(wro_sb[:], 0.0)
        nc.sync.dma_start(out=wro_sb[:, 0:2, :],
                          in_=WROIN[0:2 * P, :].rearrange("(t p) r -> p t r", p=P))
        nkm = (H + MF) - 2 * P
        full_mol_t = MF // P
        nc.sync.dma_start(
            out=wro_sb[:, 2:2 + full_mol_t, :],
            in_=WROIN[2 * P:2 * P + full_mol_t * P, :].rearrange(
                "(t p) r -> p t r", p=P))
        rem = nkm - full_mol_t * P
        if rem:
            nc.sync.dma_start(out=wro_sb[0:rem, 2 + full_mol_t, :],
                              in_=WROIN[2 * P + full_mol_t * P:, :])
        broin_sb = consts.tile([1, RH], FP32, name="broin_sb")
        nc.sync.dma_start(out=broin_sb[:], in_=BROIN[:])

        broh_sb = consts.tile([1, N_RO * RH], FP32, name="broh_sb")
        nc.sync.dma_start(out=broh_sb[:], in_=BROH[:])
        wout_sb = consts.tile([P, RT, 1], FP32, name="wout_sb")
        nc.sync.dma_start(out=wout_sb[:], in_=WOUT.rearrange("(t p) o -> p t o", p=P))
        bout_sb = consts.tile([1, 1], FP32, name="bout_sb")
        nc.sync.dma_start(out=bout_sb[:], in_=BOUT[:])
        mol_sb = consts.tile([1, MF], FP32, name="mol_sb")
        nc.sync.dma_start(out=mol_sb[:], in_=MOLV[:])
        gcat = consts.tile([P, 2 + (MF + P - 1) // P], FP32, name="gcat")

        def row_to_col(row_sb, width, out_col, col_idx):
            """scatter a [1, width] fp32 row onto partitions as [width, 1]"""
            for c in range((width + P - 1) // P):
                w = min(P, width - c * P)
                p_c = pcyc.tile([P, 1], FP32, tag="ps")
                nc.tensor.matmul(p_c[0:w, :],
                                 lhsT=row_sb[0:1, c * P:c * P + w],
                                 rhs=ones_f[0:1, 0:1], start=True, stop=True)
                nc.any.tensor_copy(out=out_col[0:w, col_idx + c:col_idx + c + 1],
                                   in_=p_c[0:w, :])

        row_to_col(mol_sb, MF, gcat, v["HC"])

        # b_att broadcast to a per-partition column
        p_b = pcyc.tile([P, 1], FP32, tag="ps")
        nc.tensor.matmul(p_b[:], lhsT=ones_f[0:1, :], rhs=batt_sb[:],
                         start=True, stop=True)
        batt_col = consts.tile([P, 1], FP32, name="batt_col")
        nc.any.tensor_copy(out=batt_col[:], in_=p_b[:])

        # W_att rows broadcast across partitions (bf16 for 2x DVE reads)
        wa_bc = []
        for a in range(2):
            p_w = pcyc.tile([P, H], FP32, tag="ps")
            nc.tensor.matmul(p_w[:], lhsT=ones_f[0:1, :],
                             rhs=watt_sb[0:1, a * H:(a + 1) * H],
                             start=True, stop=True)
            t = consts.tile([P, H], BF16, name=f"wa_bc{a}")
            nc.any.tensor_copy(out=t[:], in_=p_w[:])
            wa_bc.append(t)

        # resident cmask tiles (j-tiles 0..RES-1), loaded once
        if RES:
            cres = consts.tile([P, RES, S], FP16, name="cres")
            nc.sync.dma_start(
                out=cres[:],
                in_=CM[0:RES * P, :].rearrange("(t p) i -> p t i", p=P))

        # --------- per-tile finish: scores -> s_self row slice + s_nei col ---
        def finish_it(d_prod, it, h_all, snei_loc, srow):
            """scores for one produced h tile; transposes s_self onto srow."""
            scr = work.tile([P, H], BF16, tag="s_scr")
            scol = work.tile([P, 1], FP32, tag="scol")
            nc.vector.scalar_tensor_tensor(
                out=scr[:], in0=h_all[:, it, 0:H], scalar=1.0,
                in1=wa_bc[1][:], op0=ALU.mult, op1=ALU.mult,
                accum_out=scol[:, 0:1])
            scr2 = work.tile([P, H], BF16, tag="s_scr")
            nc.vector.scalar_tensor_tensor(
                out=scr2[:], in0=h_all[:, it, 0:H], scalar=1.0,
                in1=wa_bc[0][:], op0=ALU.mult, op1=ALU.mult,
                accum_out=snei_loc[:, it:it + 1])
            p_sc = pcyc.tile([1, P], FP32, tag="ps")
            nc.tensor.matmul(p_sc[:], lhsT=scol[:, 0:1],
                             rhs=ident_f[:], start=True, stop=True)
            nc.vector.tensor_copy(out=srow[0:1, it * P:(it + 1) * P],
                                  in_=p_sc[:])

        def finish_pay(d_prod, h_all, snei_loc):
            """fp8 payload [h | s_hi | s_lo], shipped in two halves so the
            first half's DMA overlaps the second half's conversion."""
            h8 = work.tile([P, IT, PAY], FP8, tag="h8", bufs=2)
            halves = [(0, IT // 2), (IT // 2, IT)] if IT >= 2 else [(0, IT)]
            pays_r = pays[d_prod][:].rearrange("p (t c) -> p t c", t=IT)
            for lo, hi in halves:
                nc.gpsimd.tensor_copy(out=h8[:, lo:hi, 0:H],
                                      in_=h_all[:, lo:hi, 0:H])
                nc.vector.tensor_copy(out=h8[:, lo:hi, H],
                                      in_=snei_loc[:, lo:hi])
                nc.vector.tensor_tensor(h8[:, lo:hi, H + 1],
                                        snei_loc[:, lo:hi],
                                        h8[:, lo:hi, H], ALU.subtract)
                nc.gpsimd.memset(h8[:, lo:hi, H + 2:PAY], 0.0)
                nc.sync.dma_start(out=pays_r[:, lo:hi], in_=h8[:, lo:hi])
            if v["no_collectives"]:
                nc.sync.dma_start(out=Gs[d_prod][0:P, :], in_=pays[d_prod][:])
            else:
                nc.gpsimd.collective_compute(
                    "AllGather", ALU.bypass, replica_groups=rg,
                    ins=[pays[d_prod][:].opt()], outs=[Gs[d_prod][:].opt()])

        # ---------------- input layer: h0 = X @ W_in + b_in ----------------
        h_all = big.tile([P, IT, H], BF16, tag="h_all", bufs=2)
        snei_loc = work.tile([P, IT], FP32, tag="snei_loc")
        srow = work.tile([1, S], FP32, tag="srow", bufs=2)
        for it in range(IT):
            p_h0 = pcyc.tile([P, H], FP32, tag="ps")
            nc.tensor.matmul(p_h0[:], lhsT=xt_sb[:, it * P:(it + 1) * P],
                             rhs=win_sb[:], start=True, stop=False)
            if fw2:
                nc.tensor.matmul(p_h0[:], lhsT=xt2_sb[:, it * P:(it + 1) * P],
                                 rhs=win2_sb[:], start=False, stop=False)
            nc.tensor.matmul(p_h0[:], lhsT=ones_bf[0:1, :], rhs=bin_sb[:],
                             start=False, stop=True)
            nc.vector.tensor_copy(out=h_all[:, it, :], in_=p_h0[:])
            finish_it(0, it, h_all, snei_loc, srow)
        finish_pay(0, h_all, snei_loc)

        # ---------------- GNN layers ----------------
        for d in range(DEPTH):
            # gathered payload for this layer -- one tile per source core so
            # tiles unlock as each chunk's DMA lands (head chunk only waits
            # for core 0's slice, not the whole gather)
            if IT >= 2:
                G8s = [gbuf.tile([P, IT, PAY], FP8, tag=f"g8_{c0}",
                                 name=f"g8_{d}_{c0}")
                       for c0 in range(CORES)]
                sneis = [gbuf.tile([P, IT], FP32, tag=f"sn_{c0}",
                                   name=f"sn_{d}_{c0}")
                         for c0 in range(CORES)]
                for c0 in range(CORES):
                    nc.sync.dma_start(
                        out=G8s[c0][:],
                        in_=Gs[d][c0 * P:(c0 + 1) * P, :].rearrange(
                            "p (t c) -> p t c", t=IT))
                    nc.vector.tensor_tensor(sneis[c0][:], G8s[c0][:, :, H],
                                            G8s[c0][:, :, H + 1], ALU.add)

                def snei_col(jt):
                    return sneis[jt // IT][:, jt % IT:jt % IT + 1]

                def g8_lhsT(jt0, hc):
                    return G8s[jt0 // IT][:, jt0 % IT:jt0 % IT + 2,
                                          hc * P:(hc + 1) * P]
            else:
                G8one = gbuf.tile([P, CORES * IT, PAY], FP8, tag="G8")
                sone = gbuf.tile([P, CORES * IT], FP32, tag="snei_f")
                for c0 in range(CORES):
                    nc.sync.dma_start(
                        out=G8one[:, c0 * IT:(c0 + 1) * IT, :],
                        in_=Gs[d][c0 * P:(c0 + 1) * P, :].rearrange(
                            "p (t c) -> p t c", t=IT))
                nc.vector.tensor_tensor(sone[:], G8one[:, :, H],
                                        G8one[:, :, H + 1], ALU.add)

                def snei_col(jt):
                    return sone[:, jt:jt + 1]

                def g8_lhsT(jt0, hc):
                    return G8one[:, jt0:jt0 + 2, hc * P:(hc + 1) * P]

            # s_self + b_att broadcast rows (fp16, replicated on partitions):
            # outer-product from the srow produced by the previous layer's
            # tail, with b_att folded into the psum->SBUF copy
            pre_t = work.tile([P, S], FP16, tag="pre_t", bufs=2)
            for icb in range(NIC):
                p_bc = pcyc.tile([P, IC], FP32, tag="ps")
                nc.tensor.matmul(p_bc[:], lhsT=ones_f[0:1, :],
                                 rhs=srow[0:1, icb * IC:(icb + 1) * IC],
                                 start=True, stop=True)
                nc.vector.tensor_scalar_add(pre_t[:, icb * IC:(icb + 1) * IC],
                                            p_bc[:], batt_col[:, 0:1])

            # psum accumulators for agg^T
            p_out = [[pacc.tile([P, IC], FP32, name=f"pout_{d}_{hc}_{ic}",
                                tag="pout", bufs=HC * NIC)
                      for ic in range(NIC)] for hc in range(HC)]

            # mask pipeline: rank-1 folds (DVE 2x TT + 4x TS; ~1/4 of tiles
            # on Pool) -> chunked bias-free sigmoid -> DR matmuls.
            # Head chunk uses per-tile bias'd sigmoids (no snei fold, so the
            # first matmuls start right after the AllGather lands); middle
            # runs in 8-tile chunks; a small tail chunk shortens the drain.
            if JT >= 24:
                CHS = ([(4, True), (4, False)] +
                       [(8, False)] * ((JT - 16) // 8) +
                       [(4, False), (4, False)])
            else:
                CHS = [(min(4, JT), True)]
                if JT > 4:
                    CHS.append((JT - 4, False))
            base = 0
            for csz, head in CHS:
                t2 = t2p.tile([P, SIG, S], FP16, tag="t2")
                for jl in range(csz):
                    jt = base + jl
                    if jt < RES:
                        csrc = cres[:, jt, :]
                    else:
                        sidx = jt - RES
                        if sidx % SC == 0:
                            cstr = strm.tile([P, SC, S], FP16, tag="cstr")
                            nc.sync.dma_start(
                                out=cstr[:],
                                in_=CM[jt * P:(jt + SC) * P, :].rearrange(
                                    "(t p) i -> p t i", p=P))
                        csrc = cstr[:, sidx % SC, :]
                    eng = nc.gpsimd if jt % 4 == 3 else nc.vector
                    eng.tensor_tensor(t2[:, jl, :], csrc, pre_t[:], ALU.add)
                    if not head:
                        eng.tensor_scalar_add(t2[:, jl, :], t2[:, jl, :],
                                              snei_col(jt))
                m8 = m8p.tile([P, SIG, S], FP8, tag="m8")
                if head:
                    for jl in range(csz):
                        jt = base + jl
                        nc.scalar.activation(m8[:, jl, :], t2[:, jl, :],
                                             AF.Sigmoid, bias=snei_col(jt))
                else:
                    nc.scalar.activation(m8[:, 0:csz, :], t2[:, 0:csz, :],
                                         AF.Sigmoid)
                for vq in range(csz // 2):
                    pr = base // 2 + vq
                    first = (pr == 0)
                    last = (pr == JT // 2 - 1)
                    for hc in range(HC):
                        for icb in range(NIC):
                            nc.tensor.matmul(
                                p_out[hc][icb][:],
                                lhsT=g8_lhsT(base + 2 * vq, hc),
                                rhs=m8[:, 2 * vq:2 * vq + 2,
                                       icb * IC:(icb + 1) * IC],
                                start=first, stop=last, perf_mode=DR)
                base += csz

            # agg^T -> SBUF (bf16), interleaved with W_node per i-chunk so
            # the first W_node matmuls don't wait for the whole copy
            aggT = big.tile([P, HC, S], BF16, tag="aggT", bufs=2)
            h_all = big.tile([P, IT, H], BF16, tag="h_all", bufs=2)
            if d + 1 < DEPTH:
                snei_loc = work.tile([P, IT], FP32, tag="snei_loc")
                srow = work.tile([1, S], FP32, tag="srow", bufs=2)
            for icb in range(NIC):
                for hc in range(HC):
                    nc.vector.tensor_copy(
                        out=aggT[:, hc, icb * IC:(icb + 1) * IC],
                        in_=p_out[hc][icb][:])
                # h_{d+1} = relu(agg @ W_node[d] + b_node[d])
                for it in range(icb * IT // NIC, (icb + 1) * IT // NIC):
                    p_h = pcyc.tile([P, H], FP32, tag="ps")
                    for kc in range(HC):
                        nc.tensor.matmul(p_h[:],
                                         lhsT=aggT[:, kc, it * P:(it + 1) * P],
                                         rhs=wnode_sb[:, d, kc, :],
                                         start=(kc == 0), stop=False)
                    nc.tensor.matmul(p_h[:], lhsT=ones_bf[0:1, :],
                                     rhs=bnode_sb[0:1, d * H:(d + 1) * H],
                                     start=False, stop=True)
                    nc.vector.tensor_scalar_max(h_all[:, it, :], p_h[:], 0.0)
                    if d + 1 < DEPTH:
                        finish_it(d + 1, it, h_all, snei_loc, srow)
            if d + 1 < DEPTH:
                finish_pay(d + 1, h_all, snei_loc)

        # ---------------- readout ----------------
        p_g = pacc.tile([1, H], FP32, name="p_g", tag="p_g", bufs=1)
        for it in range(IT):
            nc.tensor.matmul(p_g[:], lhsT=ones_bf[:, 0:1],
                             rhs=h_all[:, it, 0:H],
                             start=(it == 0), stop=(it == IT - 1))
        gpart = work.tile([1, H], FP32, tag="gpart")
        nc.any.tensor_copy(out=gpart[:], in_=p_g[:])
        nc.sync.dma_start(out=ar_in[:], in_=gpart[:])
        if v["no_collectives"]:
            nc.sync.dma_start(out=ar_out[:], in_=ar_in[:])
        else:
            nc.gpsimd.collective_compute("AllReduce", ALU.add, replica_groups=rg,
                                         ins=[ar_in[:].opt()],
                                         outs=[ar_out[:].opt()])
        gsum = work.tile([1, H], FP32, tag="gsum")
        nc.sync.dma_start(out=gsum[:], in_=ar_out[:])

        row_to_col(gsum, H, gcat, 0)
        kdims = [P, P] + [min(P, MF - c * P) for c in range((MF + P - 1) // P)]

        def mlp_col(col_tile, kd, w_sb, b_row):
            """column-in / column-out MLP layer (stays on partitions)."""
            p_c2 = pcyc.tile([P, RT], FP32, tag="ps")
            for nch in range(RT):
                for kt, kw in enumerate(kd):
                    nc.tensor.matmul(
                        p_c2[:, nch:nch + 1],
                        lhsT=w_sb[0:kw, kt, nch * P:(nch + 1) * P],
                        rhs=col_tile[0:kw, kt:kt + 1],
                        start=(kt == 0), stop=False)
                nc.tensor.matmul(
                    p_c2[:, nch:nch + 1],
                    lhsT=b_row[0:1, nch * P:(nch + 1) * P],
                    rhs=ones_f[0:1, 0:1], start=False, stop=True)
            g_col = work.tile([P, RT], FP32, tag="gcol")
            nc.scalar.activation(g_col[:], p_c2[:], AF.Relu)
            return g_col

        wroh_sb = t2p.tile([P, N_RO, RT, RH], FP32, tag="t2")
        nc.sync.dma_start(out=wroh_sb[:],
                          in_=WROH.rearrange("d (t p) r -> p d t r", p=P))
        gcol = mlp_col(gcat, kdims, wro_sb, broin_sb[:])
        for d2 in range(N_RO):
            gcol = mlp_col(gcol, [P] * RT, wroh_sb[:, d2],
                           broh_sb[0:1, d2 * RH:(d2 + 1) * RH])
        p_o = pcyc.tile([1, 1], FP32, tag="ps")
        for kt in range(RT):
            nc.tensor.matmul(p_o[:], lhsT=gcol[:, kt:kt + 1],
                             rhs=wout_sb[:, kt, :], start=(kt == 0), stop=False)
        nc.tensor.matmul(p_o[:], lhsT=ones_f[0:1, 0:1], rhs=bout_sb[:],
                         start=False, stop=True)
        o_sb = work.tile([1, 1], FP32, tag="o_sb")
        nc.any.tensor_copy(out=o_sb[:], in_=p_o[:])
        nc.sync.dma_start(out=OUT[:], in_=o_sb[:])


# ---------------------------------------------------------------------------
# host-side wrapper
# ---------------------------------------------------------------------------

_BUILT = {}


def _get(config):
    if config not in _BUILT:
        _BUILT[config] = build_gcnn(*config)
    return _BUILT[config]


def make_in_maps(inputs, N=8192, F=133, H=256, MF=200, RH=512, DEPTH=3,
                 N_RO=2, n_cores=8):
    import ml_dtypes
    S = N // n_cores
    f32 = lambda x: np.ascontiguousarray(np.asarray(x, dtype=np.float32))
    bf16 = lambda x: np.ascontiguousarray(
        np.asarray(x, dtype=np.float32).astype(ml_dtypes.bfloat16))
    A = np.asarray(inputs["adjacency_matrix"], dtype=np.float32)
    X = np.asarray(inputs["atom_feature_matrix"], dtype=np.float32)
    base = {
        "mol": f32(inputs["molecule_features_vector"]).reshape(1, MF),
        "w_in": bf16(inputs["W_in"]),
        "b_in": bf16(np.reshape(np.asarray(inputs["b_in"]), (1, H))),
        "w_att": f32(inputs["W_att"]).reshape(2, H),
        "b_att": f32(inputs["b_att"]).reshape(1, 1),
        "w_node": bf16(inputs["W_node"]),
        "b_node": bf16(np.reshape(np.asarray(inputs["b_node"], np.float32),
                                  (1, DEPTH * H))),
        "w_ro_in": f32(inputs["W_ro_in"]),
        "b_ro_in": f32(inputs["b_ro_in"]).reshape(1, RH),
        "w_ro_hid": f32(inputs["W_ro_hid"]),
        "b_ro_hid": f32(inputs["b_ro_hid"]).reshape(1, N_RO * RH),
        "w_out": f32(inputs["W_out"]).reshape(RH, 1),
        "b_out": f32(inputs["b_out"]).reshape(1, 1),
    }
    maps = []
    for c in range(n_cores):
        Ac = A[c * S:(c + 1) * S]
        cm = np.ascontiguousarray((Ac.T * 128.0 - 128.0).astype(np.float16))
        xt = np.ascontiguousarray(X[c * S:(c + 1) * S].T.astype(
            ml_dtypes.bfloat16))
        maps.append(dict(base, cmask=cm, atom_t=xt))
    return maps


def run(inputs, N=8192, F=133, H=256, MF=200, RH=512, DEPTH=3, N_RO=2,
        n_cores=8, **spmd_kwargs):
    nc = _get((N, F, H, MF, RH, DEPTH, N_RO, n_cores))
    in_maps = make_in_maps(inputs, N, F, H, MF, RH, DEPTH, N_RO, n_cores)
    res = run_bass_kernel_spmd(nc, in_maps, core_ids=list(range(n_cores)),
                               **spmd_kwargs)
    out = np.asarray(res.results[0]["out"], dtype=np.float32).reshape(())
    return out, res


def kernel(**inputs):
    out, _ = run(inputs)
    return out
